# revision 12
# baseline (speedup 1.0000x reference)
"""Trainium2 Bass kernel for a 2-layer GAT (PyG GATConv semantics) over a
50K-node / 1.6M-edge random graph, distributed over 8 NeuronCores.

Strategy (dst-partitioned, features replicated via AllGather):
  - core c owns destination nodes [c*NPC, (c+1)*NPC)
  - host sorts each core's dsts by padded in-degree (multiples of 4,
    descending) and packs them into blocks of 128 dsts; block K = max
    padded degree in the block, so padding waste stays ~5%
  - per-edge source rows (xw | a_s, 272B) are fetched with one indirect
    DMA per slot column from an AllGather'd per-node table; padding
    slots point at a dummy table row whose a_s = -1e30 (=> exp == 0)
  - per-dst softmax is computed without max subtraction (attention
    logits are O(1) here; matches the reference to ~1e-12)
  - aggregation = fold-tree adds over the K axis on the vector engine
  - everything downstream runs in the same block-permuted order (both
    node tables are permuted identically), so all non-gather DMAs are
    direct; the host unpermutes outputs at the end (pure reindexing)
"""

import sys

for _p in ("/opt/trn_rl_repo", "/root/.axon_site/_ro/trn_rl_repo"):
    if _p not in sys.path:
        sys.path.append(_p)

import numpy as np

import concourse.bacc as bacc
import concourse.mybir as mybir
import concourse.tile as tile
from concourse.bass import IndirectOffsetOnAxis
from concourse.bass_utils import run_bass_kernel_spmd
from concourse.masks import make_identity

F32 = mybir.dt.float32
I32 = mybir.dt.int32
AX = mybir.AxisListType
OP = mybir.AluOpType
ACT = mybir.ActivationFunctionType

NCORES = 8
NEG_BIG = -1.0e30  # a_s of the dummy row -> exp() == 0 for padding slots
NEG_SLOPE = 0.2


# ---------------------------------------------------------------- host prep
def _make_schedule(edge_index, n_nodes):
    """Sort each core's dst nodes by padded in-degree (desc), pack into
    blocks of 128, and build the (core-uniform) per-block K schedule and
    slot-source index arrays."""
    npc = n_nodes // NCORES
    src = np.asarray(edge_index[0], dtype=np.int64)
    dst = np.asarray(edge_index[1], dtype=np.int64)
    order = np.argsort(dst, kind="stable")
    ssrc_sorted = src[order]  # sources sorted by dst
    deg = np.bincount(dst, minlength=n_nodes)
    starts = np.zeros(n_nodes + 1, dtype=np.int64)
    np.cumsum(deg, out=starts[1:])

    K_of = np.maximum(1, deg).astype(np.int64)
    assert int(K_of.max()) <= 128, "max padded degree > 128 unsupported"

    nblk = -(-npc // 128)
    nrows = nblk * 128
    dummy = NCORES * nrows  # index of the dummy table row
    half = (nblk // 2) * 128  # AllGather split point (rows per core)

    perm_rows = np.full((NCORES, nrows), -1, dtype=np.int64)
    blk_k = np.zeros((NCORES, nblk), dtype=np.int64)
    for c in range(NCORES):
        lo = c * npc
        loc = np.argsort(-K_of[lo : lo + npc], kind="stable")
        perm_rows[c, :npc] = lo + loc
        kk = K_of[lo + loc]
        for b in range(nblk):
            seg = kk[b * 128 : (b + 1) * 128]
            blk_k[c, b] = int(seg.max()) if len(seg) else 4
    sched = [int(blk_k[:, b].max()) for b in range(nblk)]
    tot_slots = 128 * sum(sched)

    # pos_of[n] = permuted row index of node n within its core
    pos_of = np.zeros(n_nodes, dtype=np.int64)
    for c in range(NCORES):
        real = perm_rows[c] >= 0
        pos_of[perm_rows[c][real]] = np.nonzero(real)[0]
    # global table row of node n: the AllGather runs in two halves, so
    # the table layout is [cores x first-half | cores x second-half]
    core_of = np.arange(n_nodes) // npc
    tab_of = np.where(
        pos_of < half,
        core_of * half + pos_of,
        NCORES * half + core_of * (nrows - half) + (pos_of - half),
    )

    ssrc_cores = []
    for c in range(NCORES):
        parts = []
        for b in range(nblk):
            K = sched[b]
            nk = perm_rows[c, b * 128 : (b + 1) * 128]
            real = nk >= 0
            nn = np.where(real, nk, 0)
            p = starts[nn][:, None] + np.arange(K)[None, :]
            m = (np.arange(K)[None, :] < deg[nn][:, None]) & real[:, None]
            v = ssrc_sorted[np.clip(p, 0, len(ssrc_sorted) - 1)]
            parts.append(np.where(m, tab_of[v], dummy).reshape(-1))
        arr = np.concatenate(parts).astype(np.int32)
        assert len(arr) == tot_slots
        ssrc_cores.append(arr)

    return dict(
        npc=npc,
        sched=sched,
        nblk=nblk,
        nrows=nrows,
        half=half,
        tot_slots=tot_slots,
        perm_rows=perm_rows,
        ssrc=ssrc_cores,
    )


# ------------------------------------------------------------ device program
def _fold_add(nc, view, K, inner):
    """In-place fold-tree sum over the K axis of an SBUF AP view
    [128, K, inner]; result lands in view[:, 0, :]."""
    cur = K
    while cur > 1:
        half = 1 << (cur.bit_length() - 1)  # largest power of two <= cur
        if half == cur:
            half //= 2
        rem = cur - half  # rem <= half
        nc.vector.tensor_tensor(
            out=view[:, 0:rem, :],
            in0=view[:, 0:rem, :],
            in1=view[:, half : half + rem, :],
            op=OP.add,
        )
        cur = half


def _build_program(meta, dims):
    in_dim = dims["IN"]
    nrows, nblk = meta["nrows"], meta["nblk"]
    sched, tot_slots = meta["sched"], meta["tot_slots"]
    half = meta["half"]
    rest = nrows - half
    HC, H, C = 64, 4, 16
    FF = HC + H  # 68: xw row | a_s row
    NTAB = NCORES * nrows + 1  # + dummy row

    nc = bacc.Bacc(
        "TRN2", target_bir_lowering=False, debug=False, num_devices=NCORES
    )

    # ---- I/O (everything node-indexed is in block-permuted order)
    xT = nc.dram_tensor("xT", [in_dim, nrows], F32, kind="ExternalInput")
    ssrc = nc.dram_tensor("ssrc", [tot_slots], I32, kind="ExternalInput")
    w0_in = nc.dram_tensor("w0", [in_dim, HC], F32, kind="ExternalInput")
    w1_in = nc.dram_tensor("w1", [HC, HC], F32, kind="ExternalInput")
    lw_in = nc.dram_tensor("lw", [2 * HC, HC], F32, kind="ExternalInput")
    br = {}
    for nm in ("asb0", "adb0", "asb1", "adb1", "b0b", "b1b", "lbb"):
        br[nm] = nc.dram_tensor(nm, [128, HC], F32, kind="ExternalInput")

    h0_out = nc.dram_tensor("h0_out", [nrows, HC], F32, kind="ExternalOutput")
    h1_out = nc.dram_tensor("h1_out", [nrows, HC], F32, kind="ExternalOutput")
    lg_out = nc.dram_tensor("lg_out", [nrows, HC], F32, kind="ExternalOutput")
    pr_out = nc.dram_tensor("pr_out", [nrows, HC], F32, kind="ExternalOutput")

    # ---- internal DRAM
    xwp0_own = nc.dram_tensor("xwp0_own", [nrows, FF], F32)
    ad0_tab = nc.dram_tensor("ad0_tab", [nrows, H], F32)
    xwp0_tab = nc.dram_tensor("xwp0_tab", [NTAB, FF], F32, addr_space="Shared")
    xwp1_own = nc.dram_tensor("xwp1_own", [nrows, FF], F32)
    ad1_tab = nc.dram_tensor("ad1_tab", [nrows, H], F32)
    xwp1_tab = nc.dram_tensor("xwp1_tab", [NTAB, FF], F32, addr_space="Shared")

    with tile.TileContext(nc, num_cores=NCORES) as tc:
        with (
            tc.tile_pool(name="const", bufs=1) as cpool,
            tc.tile_pool(name="io", bufs=3) as iop,
            tc.tile_pool(name="gat", bufs=3) as gp,
            tc.tile_pool(name="small", bufs=4) as sp,
            tc.tile_pool(name="idxp", bufs=6) as ixp,
            tc.tile_pool(name="psum", bufs=2, space="PSUM") as pp,
        ):
            # constants
            ident = cpool.tile([128, 128], F32, name="ident")
            make_identity(nc, ident[:])
            w0_t = cpool.tile([in_dim, HC], F32, name="w0_t")
            nc.sync.dma_start(out=w0_t[:], in_=w0_in[:, :])
            w1_t = cpool.tile([HC, HC], F32, name="w1_t")
            nc.sync.dma_start(out=w1_t[:], in_=w1_in[:, :])
            lw_t = cpool.tile([2 * HC, HC], F32, name="lw_t")
            nc.sync.dma_start(out=lw_t[:], in_=lw_in[:, :])
            brt = {}
            for nm, hnd in br.items():
                brt[nm] = cpool.tile([128, HC], F32, name=f"{nm}_t")
                nc.sync.dma_start(out=brt[nm][:], in_=hnd[:, :])
            # dummy table row: zeros except a_s = -1e30
            drow = cpool.tile([1, FF], F32, name="drow")
            nc.vector.memset(drow[:, :HC], 0.0)
            nc.vector.memset(drow[:, HC:FF], NEG_BIG)
            nc.sync.dma_start(out=xwp0_tab[NTAB - 1 : NTAB, :], in_=drow[:, :])
            nc.sync.dma_start(out=xwp1_tab[NTAB - 1 : NTAB, :], in_=drow[:, :])

            def build_rows(src_rows_ap, kdim, asb, adb, xwp_dst, ad_dst, w_t,
                           transpose_first):
                """[128, kdim] node rows -> xw|a_s rows + a_d rows."""
                if transpose_first:
                    rows_t = iop.tile([128, kdim], F32, tag="rows_t")
                    nc.sync.dma_start(out=rows_t[:, :], in_=src_rows_ap)
                    tp = pp.tile([kdim, 128], F32, tag="tp")
                    nc.tensor.transpose(
                        out=tp[:, :], in_=rows_t[:, :], identity=ident[:]
                    )
                    lhsT = iop.tile([kdim, 128], F32, tag="lhsT")
                    nc.vector.tensor_copy(out=lhsT[:, :], in_=tp[:, :])
                else:
                    lhsT = iop.tile([kdim, 128], F32, tag="lhsT")
                    nc.sync.dma_start(out=lhsT[:, :], in_=src_rows_ap)
                xw = pp.tile([128, HC], F32, tag="xw")
                nc.tensor.matmul(
                    out=xw[:, :], lhsT=lhsT[:, :], rhs=w_t[:, :],
                    start=True, stop=True,
                )
                xwp = iop.tile([128, FF], F32, tag="xwp")
                nc.vector.tensor_copy(out=xwp[:, :HC], in_=xw[:, :])
                prod = iop.tile([128, HC], F32, tag="prod")
                nc.vector.tensor_tensor(
                    out=prod[:, :], in0=xw[:, :], in1=asb[:, :], op=OP.mult
                )
                nc.vector.tensor_reduce(
                    out=xwp[:, HC:FF],
                    in_=prod[:, :].rearrange("p (h c) -> p h c", h=H),
                    axis=AX.X,
                    op=OP.add,
                )
                nc.vector.tensor_tensor(
                    out=prod[:, :], in0=xw[:, :], in1=adb[:, :], op=OP.mult
                )
                adt = iop.tile([128, H], F32, tag="adt")
                nc.vector.tensor_reduce(
                    out=adt[:, :],
                    in_=prod[:, :].rearrange("p (h c) -> p h c", h=H),
                    axis=AX.X,
                    op=OP.add,
                )
                nc.sync.dma_start(out=xwp_dst, in_=xwp[:, :])
                nc.sync.dma_start(out=ad_dst, in_=adt[:, :])

            # ---------------- phase A: xwp0 = [x@W0 | a_s0], a_d0 (permuted)
            for i in range(nblk):
                r0 = i * 128
                build_rows(
                    xT[:, r0 : r0 + 128], in_dim, brt["asb0"], brt["adb0"],
                    xwp0_own[r0 : r0 + 128, :], ad0_tab[r0 : r0 + 128, :],
                    w0_t, transpose_first=False,
                )

            def allgather(own, tab):
                """AllGather in two halves so the first half overlaps the
                producing phase's tail; table layout is
                [cores x first-half | cores x second-half]."""
                if half > 0:
                    nc.gpsimd.collective_compute(
                        "AllGather",
                        OP.bypass,
                        replica_groups=[list(range(NCORES))],
                        ins=[own[0:half, :].opt()],
                        outs=[tab[0 : NCORES * half, :].opt()],
                    )
                nc.gpsimd.collective_compute(
                    "AllGather",
                    OP.bypass,
                    replica_groups=[list(range(NCORES))],
                    ins=[own[half:nrows, :].opt()],
                    outs=[
                        tab[NCORES * half : NCORES * half + NCORES * rest, :].opt()
                    ],
                )

            allgather(xwp0_own, xwp0_tab)

            # ---------------- edge aggregation (shared between layers)
            def edge_pass(xwp_tab_h, ad_tab_h, bias_t, h_dst):
                slot_base = 0
                for b, K in enumerate(sched):
                    r0 = b * 128
                    idx = ixp.tile([128, K], I32, tag="idx")
                    nc.sync.dma_start(
                        out=idx[:],
                        in_=ssrc[slot_base : slot_base + 128 * K].rearrange(
                            "(p k) -> p k", p=128
                        ),
                    )
                    g = gp.tile([128, K, FF], F32, tag="g", bufs=4)
                    for k in range(K):
                        nc.gpsimd.indirect_dma_start(
                            out=g[:, k, :],
                            out_offset=None,
                            in_=xwp_tab_h[:, :],
                            in_offset=IndirectOffsetOnAxis(
                                ap=idx[:, k : k + 1], axis=0
                            ),
                        )
                    ad = sp.tile([128, H], F32, tag="ad")
                    nc.sync.dma_start(out=ad[:], in_=ad_tab_h[r0 : r0 + 128, :])
                    # e = a_s[src] + a_d[dst]  -> [128, K, H]
                    e = gp.tile([128, K, H], F32, tag="e")
                    nc.vector.tensor_tensor(
                        out=e[:, :, :],
                        in0=g[:, :, HC:FF],
                        in1=ad[:, None, :].to_broadcast([128, K, H]),
                        op=OP.add,
                    )
                    # ex = exp(leaky_relu(e)) = max(exp(e), exp(0.2*e))
                    ex1 = gp.tile([128, K, H], F32, tag="ex1")
                    nc.scalar.activation(
                        out=ex1[:, :, :], in_=e[:, :, :], func=ACT.Exp
                    )
                    nc.scalar.activation(
                        out=e[:, :, :], in_=e[:, :, :], func=ACT.Exp,
                        scale=NEG_SLOPE,
                    )
                    nc.vector.tensor_tensor(
                        out=ex1[:, :, :], in0=ex1[:, :, :], in1=e[:, :, :],
                        op=OP.max,
                    )
                    # msg = xw[src] * ex  -> [128, K, H, C]
                    msg = gp.tile([128, K, H, C], F32, tag="msg")
                    nc.vector.tensor_tensor(
                        out=msg[:, :, :, :],
                        in0=g[:, :, :HC].rearrange("p k (h c) -> p k h c", h=H),
                        in1=ex1[:, :, :, None].to_broadcast([128, K, H, C]),
                        op=OP.mult,
                    )
                    _fold_add(nc, ex1[:, :, :], K, H)
                    _fold_add(nc, msg[:].rearrange("p k h c -> p k (h c)"), K, HC)
                    rden = sp.tile([128, H], F32, tag="rden")
                    nc.vector.tensor_scalar_add(rden[:, :], ex1[:, 0, :], 1e-16)
                    nc.vector.reciprocal(rden[:, :], rden[:, :])
                    outt = sp.tile([128, HC], F32, tag="outt")
                    nc.vector.tensor_tensor(
                        out=outt[:].rearrange("p (h c) -> p h c", h=H),
                        in0=msg[:, 0, :, :],
                        in1=rden[:, :, None].to_broadcast([128, H, C]),
                        op=OP.mult,
                    )
                    nc.vector.tensor_tensor(
                        out=outt[:, :], in0=outt[:, :], in1=bias_t[:, :],
                        op=OP.add,
                    )
                    # elu(x) = max(x,0) + exp(min(x,0)) - 1
                    mn = sp.tile([128, HC], F32, tag="mn")
                    nc.vector.tensor_scalar_min(mn[:, :], outt[:, :], 0.0)
                    nc.scalar.activation(out=mn[:, :], in_=mn[:, :], func=ACT.Exp)
                    nc.vector.tensor_scalar_max(outt[:, :], outt[:, :], 0.0)
                    nc.vector.tensor_tensor(
                        out=outt[:, :], in0=outt[:, :], in1=mn[:, :], op=OP.add
                    )
                    ht = sp.tile([128, HC], F32, tag="ht")
                    nc.vector.tensor_scalar_add(ht[:, :], outt[:, :], -1.0)
                    nc.sync.dma_start(out=h_dst[r0 : r0 + 128, :], in_=ht[:, :])
                    slot_base += 128 * K

            edge_pass(xwp0_tab, ad0_tab, brt["b0b"], h0_out)

            # ---------------- phase C: xwp1 = [h0@W1 | a_s1], a_d1 (permuted)
            for i in range(nblk):
                r0 = i * 128
                build_rows(
                    h0_out[r0 : r0 + 128, :], HC, brt["asb1"], brt["adb1"],
                    xwp1_own[r0 : r0 + 128, :], ad1_tab[r0 : r0 + 128, :],
                    w1_t, transpose_first=True,
                )

            allgather(xwp1_own, xwp1_tab)

            edge_pass(xwp1_tab, ad1_tab, brt["b1b"], h1_out)

            # ---------------- phase D: logits + softmax (permuted order)
            for i in range(nblk):
                r0 = i * 128
                embT = iop.tile([128, 128], F32, tag="embT")
                for j, h_src in enumerate((h0_out, h1_out)):
                    hc_t = iop.tile([128, HC], F32, tag="hc_t")
                    nc.sync.dma_start(out=hc_t[:], in_=h_src[r0 : r0 + 128, :])
                    tp = pp.tile([HC, 128], F32, tag="tp")
                    nc.tensor.transpose(
                        out=tp[:, :], in_=hc_t[:, :], identity=ident[:]
                    )
                    nc.vector.tensor_copy(
                        out=embT[j * HC : (j + 1) * HC, :], in_=tp[:, :]
                    )
                lg_p = pp.tile([128, HC], F32, tag="lg_p")
                nc.tensor.matmul(
                    out=lg_p[:, :], lhsT=embT[:, :], rhs=lw_t[:, :],
                    start=True, stop=True,
                )
                lg = sp.tile([128, HC], F32, tag="lg")
                nc.vector.tensor_tensor(
                    out=lg[:, :], in0=lg_p[:, :], in1=brt["lbb"][:, :], op=OP.add
                )
                nc.sync.dma_start(out=lg_out[r0 : r0 + 128, :], in_=lg[:, :])
                mx = sp.tile([128, 1], F32, tag="mx")
                nc.vector.tensor_reduce(
                    out=mx[:, :], in_=lg[:, :], axis=AX.X, op=OP.max
                )
                pr = sp.tile([128, HC], F32, tag="pr")
                nc.vector.tensor_scalar(
                    out=pr[:, :], in0=lg[:, :], scalar1=mx[:, :1], scalar2=None,
                    op0=OP.subtract,
                )
                nc.scalar.activation(out=pr[:, :], in_=pr[:, :], func=ACT.Exp)
                sm = sp.tile([128, 1], F32, tag="sm")
                nc.vector.tensor_reduce(
                    out=sm[:, :], in_=pr[:, :], axis=AX.X, op=OP.add
                )
                nc.vector.reciprocal(sm[:, :], sm[:, :])
                nc.vector.tensor_scalar(
                    out=pr[:, :], in0=pr[:, :], scalar1=sm[:, :1], scalar2=None,
                    op0=OP.mult,
                )
                nc.sync.dma_start(out=pr_out[r0 : r0 + 128, :], in_=pr[:, :])

    nc.compile()
    return nc


# ----------------------------------------------------------------- execution
def run_gat(x, edge_index, W0, as0, ad0, b0, W1, as1, ad1, b1, lw, lb,
            trace=False):
    n_nodes, in_dim = x.shape
    meta = _make_schedule(edge_index, n_nodes)
    nc = _build_program(meta, {"N": n_nodes, "IN": in_dim})

    nrows = meta["nrows"]
    rep = lambda v: np.tile(np.asarray(v, np.float32).reshape(1, -1), (128, 1))
    x = np.asarray(x, np.float32)
    in_maps = []
    for c in range(NCORES):
        rows = meta["perm_rows"][c]
        xp = np.zeros((nrows, in_dim), np.float32)
        real = rows >= 0
        xp[real] = x[rows[real]]
        in_maps.append(
            {
                "xT": np.ascontiguousarray(xp.T),
                "ssrc": meta["ssrc"][c],
                "w0": np.asarray(W0, np.float32),
                "w1": np.asarray(W1, np.float32),
                "lw": np.asarray(lw, np.float32),
                "asb0": rep(as0),
                "adb0": rep(ad0),
                "asb1": rep(as1),
                "adb1": rep(ad1),
                "b0b": rep(b0),
                "b1b": rep(b1),
                "lbb": rep(lb),
            }
        )

    res = None
    for attempt in range(3):
        try:
            res = run_bass_kernel_spmd(
                nc, in_maps, core_ids=list(range(NCORES)), trace=trace
            )
            break
        except Exception:
            if attempt == 2:
                raise
            import time as _time

            _time.sleep(5.0)

    HC = 64
    h0 = np.empty((n_nodes, HC), np.float32)
    h1 = np.empty((n_nodes, HC), np.float32)
    lg = np.empty((n_nodes, HC), np.float32)
    pr = np.empty((n_nodes, HC), np.float32)
    for c in range(NCORES):
        rows = meta["perm_rows"][c]
        real = rows >= 0
        nid = rows[real]
        pos = np.nonzero(real)[0]
        r = res.results[c]
        h0[nid] = r["h0_out"][pos]
        h1[nid] = r["h1_out"][pos]
        lg[nid] = r["lg_out"][pos]
        pr[nid] = r["pr_out"][pos]

    views = np.stack([h0, h1])
    final_emb = np.concatenate([h0, h1], axis=1)
    return (views, final_emb, lg, pr), res


def kernel(**inputs):
    outs, _ = run_gat(
        inputs["x"], inputs["edge_index"],
        inputs["W0"], inputs["as0"], inputs["ad0"], inputs["b0"],
        inputs["W1"], inputs["as1"], inputs["ad1"], inputs["b1"],
        inputs["lw"], inputs["lb"],
    )
    return outs


# revision 14
# speedup vs baseline: 1.0036x; 1.0036x over previous
"""Trainium2 Bass kernel for a 2-layer GAT (PyG GATConv semantics) over a
50K-node / 1.6M-edge random graph, distributed over 8 NeuronCores.

Strategy (dst-partitioned, features replicated via AllGather):
  - core c owns destination nodes [c*NPC, (c+1)*NPC)
  - host sorts each core's dsts by padded in-degree (multiples of 4,
    descending) and packs them into blocks of 128 dsts; block K = max
    padded degree in the block, so padding waste stays ~5%
  - per-edge source rows (xw | a_s, 272B) are fetched with one indirect
    DMA per slot column from an AllGather'd per-node table; padding
    slots point at a dummy table row whose a_s = -1e30 (=> exp == 0)
  - per-dst softmax is computed without max subtraction (attention
    logits are O(1) here; matches the reference to ~1e-12)
  - aggregation = fold-tree adds over the K axis on the vector engine
  - everything downstream runs in the same block-permuted order (both
    node tables are permuted identically), so all non-gather DMAs are
    direct; the host unpermutes outputs at the end (pure reindexing)
"""

import sys

for _p in ("/opt/trn_rl_repo", "/root/.axon_site/_ro/trn_rl_repo"):
    if _p not in sys.path:
        sys.path.append(_p)

import numpy as np

import concourse.bacc as bacc
import concourse.mybir as mybir
import concourse.tile as tile
from concourse.bass import IndirectOffsetOnAxis
from concourse.bass_utils import run_bass_kernel_spmd
from concourse.masks import make_identity

# If the container's antenv stub lacks axon_hooks, BASS_TRACE=1 would make
# run_bass_kernel_spmd crash on import; register a no-op hook module so
# tracing degrades gracefully instead.
try:
    from antenv import axon_hooks as _axon_hooks  # noqa: F401
except ImportError:
    import types as _types

    _m = _types.ModuleType("antenv.axon_hooks")
    _m.get_axon_ntff_profile_hook = lambda: None
    _m.set_axon_ntff_profile_hook = lambda h: None
    sys.modules["antenv.axon_hooks"] = _m

F32 = mybir.dt.float32
I32 = mybir.dt.int32
AX = mybir.AxisListType
OP = mybir.AluOpType
ACT = mybir.ActivationFunctionType

NCORES = 8
NEG_BIG = -1.0e30  # a_s of the dummy row -> exp() == 0 for padding slots
NEG_SLOPE = 0.2


# ---------------------------------------------------------------- host prep
def _make_schedule(edge_index, n_nodes):
    """Sort each core's dst nodes by padded in-degree (desc), pack into
    blocks of 128, and build the (core-uniform) per-block K schedule and
    slot-source index arrays."""
    npc = n_nodes // NCORES
    src = np.asarray(edge_index[0], dtype=np.int64)
    dst = np.asarray(edge_index[1], dtype=np.int64)
    order = np.argsort(dst, kind="stable")
    ssrc_sorted = src[order]  # sources sorted by dst
    deg = np.bincount(dst, minlength=n_nodes)
    starts = np.zeros(n_nodes + 1, dtype=np.int64)
    np.cumsum(deg, out=starts[1:])

    K_of = np.maximum(1, deg).astype(np.int64)
    assert int(K_of.max()) <= 128, "max padded degree > 128 unsupported"

    nblk = -(-npc // 128)
    nrows = nblk * 128
    dummy = NCORES * nrows  # index of the dummy table row
    half = (nblk // 2) * 128  # AllGather split point (rows per core)

    perm_rows = np.full((NCORES, nrows), -1, dtype=np.int64)
    blk_k = np.zeros((NCORES, nblk), dtype=np.int64)
    for c in range(NCORES):
        lo = c * npc
        loc = np.argsort(-K_of[lo : lo + npc], kind="stable")
        perm_rows[c, :npc] = lo + loc
        kk = K_of[lo + loc]
        for b in range(nblk):
            seg = kk[b * 128 : (b + 1) * 128]
            blk_k[c, b] = int(seg.max()) if len(seg) else 4
    sched = [int(blk_k[:, b].max()) for b in range(nblk)]
    tot_slots = 128 * sum(sched)

    # pos_of[n] = permuted row index of node n within its core
    pos_of = np.zeros(n_nodes, dtype=np.int64)
    for c in range(NCORES):
        real = perm_rows[c] >= 0
        pos_of[perm_rows[c][real]] = np.nonzero(real)[0]
    # global table row of node n: the AllGather runs in two halves, so
    # the table layout is [cores x first-half | cores x second-half]
    core_of = np.arange(n_nodes) // npc
    tab_of = np.where(
        pos_of < half,
        core_of * half + pos_of,
        NCORES * half + core_of * (nrows - half) + (pos_of - half),
    )

    ssrc_cores = []
    for c in range(NCORES):
        parts = []
        for b in range(nblk):
            K = sched[b]
            nk = perm_rows[c, b * 128 : (b + 1) * 128]
            real = nk >= 0
            nn = np.where(real, nk, 0)
            p = starts[nn][:, None] + np.arange(K)[None, :]
            m = (np.arange(K)[None, :] < deg[nn][:, None]) & real[:, None]
            v = ssrc_sorted[np.clip(p, 0, len(ssrc_sorted) - 1)]
            parts.append(np.where(m, tab_of[v], dummy).reshape(-1))
        arr = np.concatenate(parts).astype(np.int32)
        assert len(arr) == tot_slots
        ssrc_cores.append(arr)

    return dict(
        npc=npc,
        sched=sched,
        nblk=nblk,
        nrows=nrows,
        half=half,
        tot_slots=tot_slots,
        perm_rows=perm_rows,
        ssrc=ssrc_cores,
    )


# ------------------------------------------------------------ device program
def _fold_add(nc, view, K, inner):
    """In-place fold-tree sum over the K axis of an SBUF AP view
    [128, K, inner]; result lands in view[:, 0, :]."""
    cur = K
    while cur > 1:
        half = 1 << (cur.bit_length() - 1)  # largest power of two <= cur
        if half == cur:
            half //= 2
        rem = cur - half  # rem <= half
        nc.vector.tensor_tensor(
            out=view[:, 0:rem, :],
            in0=view[:, 0:rem, :],
            in1=view[:, half : half + rem, :],
            op=OP.add,
        )
        cur = half


def _build_program(meta, dims):
    in_dim = dims["IN"]
    nrows, nblk = meta["nrows"], meta["nblk"]
    sched, tot_slots = meta["sched"], meta["tot_slots"]
    half = meta["half"]
    rest = nrows - half
    HC, H, C = 64, 4, 16
    FF = HC + H  # 68: xw row | a_s row
    NTAB = NCORES * nrows + 1  # + dummy row

    nc = bacc.Bacc(
        "TRN2", target_bir_lowering=False, debug=False, num_devices=NCORES
    )

    # ---- I/O (everything node-indexed is in block-permuted order)
    xT = nc.dram_tensor("xT", [in_dim, nrows], F32, kind="ExternalInput")
    ssrc = nc.dram_tensor("ssrc", [tot_slots], I32, kind="ExternalInput")
    w0_in = nc.dram_tensor("w0", [in_dim, HC], F32, kind="ExternalInput")
    w1_in = nc.dram_tensor("w1", [HC, HC], F32, kind="ExternalInput")
    lw_in = nc.dram_tensor("lw", [2 * HC, HC], F32, kind="ExternalInput")
    br = {}
    for nm in ("asb0", "adb0", "asb1", "adb1", "b0b", "b1b", "lbb"):
        br[nm] = nc.dram_tensor(nm, [128, HC], F32, kind="ExternalInput")

    h0_out = nc.dram_tensor("h0_out", [nrows, HC], F32, kind="ExternalOutput")
    h1_out = nc.dram_tensor("h1_out", [nrows, HC], F32, kind="ExternalOutput")
    lg_out = nc.dram_tensor("lg_out", [nrows, HC], F32, kind="ExternalOutput")
    pr_out = nc.dram_tensor("pr_out", [nrows, HC], F32, kind="ExternalOutput")

    # ---- internal DRAM
    xwp0_own = nc.dram_tensor("xwp0_own", [nrows, FF], F32)
    ad0_tab = nc.dram_tensor("ad0_tab", [nrows, H], F32)
    xwp0_tab = nc.dram_tensor("xwp0_tab", [NTAB, FF], F32, addr_space="Shared")
    xwp1_own = nc.dram_tensor("xwp1_own", [nrows, FF], F32)
    ad1_tab = nc.dram_tensor("ad1_tab", [nrows, H], F32)
    xwp1_tab = nc.dram_tensor("xwp1_tab", [NTAB, FF], F32, addr_space="Shared")

    with tile.TileContext(nc, num_cores=NCORES) as tc:
        with (
            tc.tile_pool(name="const", bufs=1) as cpool,
            tc.tile_pool(name="io", bufs=3) as iop,
            tc.tile_pool(name="gat", bufs=3) as gp,
            tc.tile_pool(name="small", bufs=4) as sp,
            tc.tile_pool(name="idxp", bufs=6) as ixp,
            tc.tile_pool(name="psum", bufs=2, space="PSUM") as pp,
        ):
            # constants
            ident = cpool.tile([128, 128], F32, name="ident")
            make_identity(nc, ident[:])
            w0_t = cpool.tile([in_dim, HC], F32, name="w0_t")
            nc.sync.dma_start(out=w0_t[:], in_=w0_in[:, :])
            w1_t = cpool.tile([HC, HC], F32, name="w1_t")
            nc.sync.dma_start(out=w1_t[:], in_=w1_in[:, :])
            lw_t = cpool.tile([2 * HC, HC], F32, name="lw_t")
            nc.sync.dma_start(out=lw_t[:], in_=lw_in[:, :])
            brt = {}
            for nm, hnd in br.items():
                brt[nm] = cpool.tile([128, HC], F32, name=f"{nm}_t")
                nc.sync.dma_start(out=brt[nm][:], in_=hnd[:, :])
            # dummy table row: zeros except a_s = -1e30
            drow = cpool.tile([1, FF], F32, name="drow")
            nc.vector.memset(drow[:, :HC], 0.0)
            nc.vector.memset(drow[:, HC:FF], NEG_BIG)
            nc.sync.dma_start(out=xwp0_tab[NTAB - 1 : NTAB, :], in_=drow[:, :])
            nc.sync.dma_start(out=xwp1_tab[NTAB - 1 : NTAB, :], in_=drow[:, :])

            def build_rows(src_rows_ap, kdim, asb, adb, xwp_dst, ad_dst, w_t,
                           transpose_first):
                """[128, kdim] node rows -> xw|a_s rows + a_d rows."""
                if transpose_first:
                    rows_t = iop.tile([128, kdim], F32, tag="rows_t")
                    nc.sync.dma_start(out=rows_t[:, :], in_=src_rows_ap)
                    tp = pp.tile([kdim, 128], F32, tag="tp")
                    nc.tensor.transpose(
                        out=tp[:, :], in_=rows_t[:, :], identity=ident[:]
                    )
                    lhsT = iop.tile([kdim, 128], F32, tag="lhsT")
                    nc.vector.tensor_copy(out=lhsT[:, :], in_=tp[:, :])
                else:
                    lhsT = iop.tile([kdim, 128], F32, tag="lhsT")
                    nc.sync.dma_start(out=lhsT[:, :], in_=src_rows_ap)
                xw = pp.tile([128, HC], F32, tag="xw")
                nc.tensor.matmul(
                    out=xw[:, :], lhsT=lhsT[:, :], rhs=w_t[:, :],
                    start=True, stop=True,
                )
                xwp = iop.tile([128, FF], F32, tag="xwp")
                nc.vector.tensor_copy(out=xwp[:, :HC], in_=xw[:, :])
                prod = iop.tile([128, HC], F32, tag="prod")
                nc.vector.tensor_tensor(
                    out=prod[:, :], in0=xw[:, :], in1=asb[:, :], op=OP.mult
                )
                nc.vector.tensor_reduce(
                    out=xwp[:, HC:FF],
                    in_=prod[:, :].rearrange("p (h c) -> p h c", h=H),
                    axis=AX.X,
                    op=OP.add,
                )
                nc.vector.tensor_tensor(
                    out=prod[:, :], in0=xw[:, :], in1=adb[:, :], op=OP.mult
                )
                adt = iop.tile([128, H], F32, tag="adt")
                nc.vector.tensor_reduce(
                    out=adt[:, :],
                    in_=prod[:, :].rearrange("p (h c) -> p h c", h=H),
                    axis=AX.X,
                    op=OP.add,
                )
                nc.sync.dma_start(out=xwp_dst, in_=xwp[:, :])
                nc.sync.dma_start(out=ad_dst, in_=adt[:, :])

            # ---------------- phase A: xwp0 = [x@W0 | a_s0], a_d0 (permuted)
            for i in range(nblk):
                r0 = i * 128
                build_rows(
                    xT[:, r0 : r0 + 128], in_dim, brt["asb0"], brt["adb0"],
                    xwp0_own[r0 : r0 + 128, :], ad0_tab[r0 : r0 + 128, :],
                    w0_t, transpose_first=False,
                )

            def allgather(own, tab):
                """AllGather in two halves so the first half overlaps the
                producing phase's tail; table layout is
                [cores x first-half | cores x second-half]."""
                if half > 0:
                    nc.gpsimd.collective_compute(
                        "AllGather",
                        OP.bypass,
                        replica_groups=[list(range(NCORES))],
                        ins=[own[0:half, :].opt()],
                        outs=[tab[0 : NCORES * half, :].opt()],
                    )
                nc.gpsimd.collective_compute(
                    "AllGather",
                    OP.bypass,
                    replica_groups=[list(range(NCORES))],
                    ins=[own[half:nrows, :].opt()],
                    outs=[
                        tab[NCORES * half : NCORES * half + NCORES * rest, :].opt()
                    ],
                )

            allgather(xwp0_own, xwp0_tab)

            # ---------------- edge aggregation (shared between layers)
            def edge_pass(xwp_tab_h, ad_tab_h, bias_t, h_dst):
                slot_base = 0
                for b, K in enumerate(sched):
                    r0 = b * 128
                    idx = ixp.tile([128, K], I32, tag="idx")
                    nc.sync.dma_start(
                        out=idx[:],
                        in_=ssrc[slot_base : slot_base + 128 * K].rearrange(
                            "(p k) -> p k", p=128
                        ),
                    )
                    g = gp.tile([128, K, FF], F32, tag="g", bufs=4)
                    for k in range(K):
                        nc.gpsimd.indirect_dma_start(
                            out=g[:, k, :],
                            out_offset=None,
                            in_=xwp_tab_h[:, :],
                            in_offset=IndirectOffsetOnAxis(
                                ap=idx[:, k : k + 1], axis=0
                            ),
                        )
                    ad = sp.tile([128, H], F32, tag="ad")
                    nc.sync.dma_start(out=ad[:], in_=ad_tab_h[r0 : r0 + 128, :])
                    # e = a_s[src] + a_d[dst]  -> [128, K, H]
                    e = gp.tile([128, K, H], F32, tag="e")
                    nc.vector.tensor_tensor(
                        out=e[:, :, :],
                        in0=g[:, :, HC:FF],
                        in1=ad[:, None, :].to_broadcast([128, K, H]),
                        op=OP.add,
                    )
                    # ex = exp(leaky_relu(e)) = max(exp(e), exp(0.2*e))
                    ex1 = gp.tile([128, K, H], F32, tag="ex1")
                    nc.scalar.activation(
                        out=ex1[:, :, :], in_=e[:, :, :], func=ACT.Exp
                    )
                    nc.scalar.activation(
                        out=e[:, :, :], in_=e[:, :, :], func=ACT.Exp,
                        scale=NEG_SLOPE,
                    )
                    nc.vector.tensor_tensor(
                        out=ex1[:, :, :], in0=ex1[:, :, :], in1=e[:, :, :],
                        op=OP.max,
                    )
                    # msg = xw[src] * ex  -> [128, K, H, C]
                    msg = gp.tile([128, K, H, C], F32, tag="msg")
                    nc.vector.tensor_tensor(
                        out=msg[:, :, :, :],
                        in0=g[:, :, :HC].rearrange("p k (h c) -> p k h c", h=H),
                        in1=ex1[:, :, :, None].to_broadcast([128, K, H, C]),
                        op=OP.mult,
                    )
                    _fold_add(nc, ex1[:, :, :], K, H)
                    _fold_add(nc, msg[:].rearrange("p k h c -> p k (h c)"), K, HC)
                    rden = sp.tile([128, H], F32, tag="rden")
                    nc.vector.tensor_scalar_add(rden[:, :], ex1[:, 0, :], 1e-16)
                    nc.vector.reciprocal(rden[:, :], rden[:, :])
                    outt = sp.tile([128, HC], F32, tag="outt")
                    nc.vector.tensor_tensor(
                        out=outt[:].rearrange("p (h c) -> p h c", h=H),
                        in0=msg[:, 0, :, :],
                        in1=rden[:, :, None].to_broadcast([128, H, C]),
                        op=OP.mult,
                    )
                    nc.vector.tensor_tensor(
                        out=outt[:, :], in0=outt[:, :], in1=bias_t[:, :],
                        op=OP.add,
                    )
                    # elu(x) = max(x,0) + exp(min(x,0)) - 1
                    mn = sp.tile([128, HC], F32, tag="mn")
                    nc.vector.tensor_scalar_min(mn[:, :], outt[:, :], 0.0)
                    nc.scalar.activation(out=mn[:, :], in_=mn[:, :], func=ACT.Exp)
                    nc.vector.tensor_scalar_max(outt[:, :], outt[:, :], 0.0)
                    nc.vector.tensor_tensor(
                        out=outt[:, :], in0=outt[:, :], in1=mn[:, :], op=OP.add
                    )
                    ht = sp.tile([128, HC], F32, tag="ht")
                    nc.vector.tensor_scalar_add(ht[:, :], outt[:, :], -1.0)
                    nc.sync.dma_start(out=h_dst[r0 : r0 + 128, :], in_=ht[:, :])
                    slot_base += 128 * K

            edge_pass(xwp0_tab, ad0_tab, brt["b0b"], h0_out)

            # ---------------- phase C: xwp1 = [h0@W1 | a_s1], a_d1 (permuted)
            for i in range(nblk):
                r0 = i * 128
                build_rows(
                    h0_out[r0 : r0 + 128, :], HC, brt["asb1"], brt["adb1"],
                    xwp1_own[r0 : r0 + 128, :], ad1_tab[r0 : r0 + 128, :],
                    w1_t, transpose_first=True,
                )

            allgather(xwp1_own, xwp1_tab)

            edge_pass(xwp1_tab, ad1_tab, brt["b1b"], h1_out)

            # ---------------- phase D: logits + softmax (permuted order)
            for i in range(nblk):
                r0 = i * 128
                embT = iop.tile([128, 128], F32, tag="embT")
                for j, h_src in enumerate((h0_out, h1_out)):
                    hc_t = iop.tile([128, HC], F32, tag="hc_t")
                    nc.sync.dma_start(out=hc_t[:], in_=h_src[r0 : r0 + 128, :])
                    tp = pp.tile([HC, 128], F32, tag="tp")
                    nc.tensor.transpose(
                        out=tp[:, :], in_=hc_t[:, :], identity=ident[:]
                    )
                    nc.vector.tensor_copy(
                        out=embT[j * HC : (j + 1) * HC, :], in_=tp[:, :]
                    )
                lg_p = pp.tile([128, HC], F32, tag="lg_p")
                nc.tensor.matmul(
                    out=lg_p[:, :], lhsT=embT[:, :], rhs=lw_t[:, :],
                    start=True, stop=True,
                )
                lg = sp.tile([128, HC], F32, tag="lg")
                nc.vector.tensor_tensor(
                    out=lg[:, :], in0=lg_p[:, :], in1=brt["lbb"][:, :], op=OP.add
                )
                nc.sync.dma_start(out=lg_out[r0 : r0 + 128, :], in_=lg[:, :])
                mx = sp.tile([128, 1], F32, tag="mx")
                nc.vector.tensor_reduce(
                    out=mx[:, :], in_=lg[:, :], axis=AX.X, op=OP.max
                )
                pr = sp.tile([128, HC], F32, tag="pr")
                nc.vector.tensor_scalar(
                    out=pr[:, :], in0=lg[:, :], scalar1=mx[:, :1], scalar2=None,
                    op0=OP.subtract,
                )
                nc.scalar.activation(out=pr[:, :], in_=pr[:, :], func=ACT.Exp)
                sm = sp.tile([128, 1], F32, tag="sm")
                nc.vector.tensor_reduce(
                    out=sm[:, :], in_=pr[:, :], axis=AX.X, op=OP.add
                )
                nc.vector.reciprocal(sm[:, :], sm[:, :])
                nc.vector.tensor_scalar(
                    out=pr[:, :], in0=pr[:, :], scalar1=sm[:, :1], scalar2=None,
                    op0=OP.mult,
                )
                nc.sync.dma_start(out=pr_out[r0 : r0 + 128, :], in_=pr[:, :])

    nc.compile()
    return nc


# ----------------------------------------------------------------- execution
_PROGRAM_CACHE = {}


def _get_program(edge_index, n_nodes, in_dim):
    import hashlib

    ei = np.ascontiguousarray(np.asarray(edge_index, np.int32))
    key = (n_nodes, in_dim, hashlib.sha1(ei.tobytes()).hexdigest())
    if key not in _PROGRAM_CACHE:
        meta = _make_schedule(ei, n_nodes)
        nc = _build_program(meta, {"N": n_nodes, "IN": in_dim})
        _PROGRAM_CACHE[key] = (meta, nc)
    return _PROGRAM_CACHE[key]


def run_gat(x, edge_index, W0, as0, ad0, b0, W1, as1, ad1, b1, lw, lb,
            trace=False):
    n_nodes, in_dim = x.shape
    meta, nc = _get_program(edge_index, n_nodes, in_dim)

    nrows = meta["nrows"]
    rep = lambda v: np.tile(np.asarray(v, np.float32).reshape(1, -1), (128, 1))
    x = np.asarray(x, np.float32)
    in_maps = []
    for c in range(NCORES):
        rows = meta["perm_rows"][c]
        xp = np.zeros((nrows, in_dim), np.float32)
        real = rows >= 0
        xp[real] = x[rows[real]]
        in_maps.append(
            {
                "xT": np.ascontiguousarray(xp.T),
                "ssrc": meta["ssrc"][c],
                "w0": np.asarray(W0, np.float32),
                "w1": np.asarray(W1, np.float32),
                "lw": np.asarray(lw, np.float32),
                "asb0": rep(as0),
                "adb0": rep(ad0),
                "asb1": rep(as1),
                "adb1": rep(ad1),
                "b0b": rep(b0),
                "b1b": rep(b1),
                "lbb": rep(lb),
            }
        )

    res = None
    for attempt in range(3):
        try:
            res = run_bass_kernel_spmd(
                nc, in_maps, core_ids=list(range(NCORES)), trace=trace
            )
            break
        except Exception:
            if attempt == 2:
                raise
            import time as _time

            _time.sleep(5.0)

    HC = 64
    h0 = np.empty((n_nodes, HC), np.float32)
    h1 = np.empty((n_nodes, HC), np.float32)
    lg = np.empty((n_nodes, HC), np.float32)
    pr = np.empty((n_nodes, HC), np.float32)
    for c in range(NCORES):
        rows = meta["perm_rows"][c]
        real = rows >= 0
        nid = rows[real]
        pos = np.nonzero(real)[0]
        r = res.results[c]
        h0[nid] = r["h0_out"][pos]
        h1[nid] = r["h1_out"][pos]
        lg[nid] = r["lg_out"][pos]
        pr[nid] = r["pr_out"][pos]

    views = np.stack([h0, h1])
    final_emb = np.concatenate([h0, h1], axis=1)
    return (views, final_emb, lg, pr), res


def kernel(**inputs):
    outs, _ = run_gat(
        inputs["x"], inputs["edge_index"],
        inputs["W0"], inputs["as0"], inputs["ad0"], inputs["b0"],
        inputs["W1"], inputs["as1"], inputs["ad1"], inputs["b1"],
        inputs["lw"], inputs["lb"],
    )
    return outs


# revision 17
# speedup vs baseline: 1.0115x; 1.0079x over previous
"""Trainium2 Bass kernel for a 2-layer GAT (PyG GATConv semantics) over a
50K-node / 1.6M-edge random graph, distributed over 8 NeuronCores.

Strategy (dst-partitioned, features replicated via AllGather):
  - core c owns destination nodes [c*NPC, (c+1)*NPC)
  - host sorts each core's dsts by padded in-degree (multiples of 4,
    descending) and packs them into blocks of 128 dsts; block K = max
    padded degree in the block, so padding waste stays ~5%
  - per-edge source rows (xw | a_s, 272B) are fetched with one indirect
    DMA per slot column from an AllGather'd per-node table; padding
    slots point at a dummy table row whose a_s = -1e30 (=> exp == 0)
  - per-dst softmax is computed without max subtraction (attention
    logits are O(1) here; matches the reference to ~1e-12)
  - aggregation = fold-tree adds over the K axis on the vector engine
  - everything downstream runs in the same block-permuted order (both
    node tables are permuted identically), so all non-gather DMAs are
    direct; the host unpermutes outputs at the end (pure reindexing)
"""

import sys

for _p in ("/opt/trn_rl_repo", "/root/.axon_site/_ro/trn_rl_repo"):
    if _p not in sys.path:
        sys.path.append(_p)

import numpy as np

import concourse.bacc as bacc
import concourse.mybir as mybir
import concourse.tile as tile
from concourse.bass import IndirectOffsetOnAxis
from concourse.bass_utils import run_bass_kernel_spmd
from concourse.masks import make_identity

# If the container's antenv stub lacks axon_hooks, BASS_TRACE=1 would make
# run_bass_kernel_spmd crash on import; register a no-op hook module so
# tracing degrades gracefully instead.
try:
    from antenv import axon_hooks as _axon_hooks  # noqa: F401
except ImportError:
    import types as _types

    _m = _types.ModuleType("antenv.axon_hooks")
    _m.get_axon_ntff_profile_hook = lambda: None
    _m.set_axon_ntff_profile_hook = lambda h: None
    sys.modules["antenv.axon_hooks"] = _m

F32 = mybir.dt.float32
I32 = mybir.dt.int32
AX = mybir.AxisListType
OP = mybir.AluOpType
ACT = mybir.ActivationFunctionType

NCORES = 8
NEG_BIG = -1.0e30  # a_s of the dummy row -> exp() == 0 for padding slots
NEG_SLOPE = 0.2


# ---------------------------------------------------------------- host prep
def _make_schedule(edge_index, n_nodes):
    """Sort each core's dst nodes by padded in-degree (desc), pack into
    blocks of 128, and build the (core-uniform) per-block K schedule and
    slot-source index arrays."""
    npc = n_nodes // NCORES
    src = np.asarray(edge_index[0], dtype=np.int64)
    dst = np.asarray(edge_index[1], dtype=np.int64)
    order = np.argsort(dst, kind="stable")
    ssrc_sorted = src[order]  # sources sorted by dst
    deg = np.bincount(dst, minlength=n_nodes)
    starts = np.zeros(n_nodes + 1, dtype=np.int64)
    np.cumsum(deg, out=starts[1:])

    K_of = np.maximum(1, deg).astype(np.int64)
    assert int(K_of.max()) <= 128, "max padded degree > 128 unsupported"

    nblk = -(-npc // 128)
    nrows = nblk * 128
    dummy = NCORES * nrows  # index of the dummy table row
    half = (nblk // 2) * 128  # AllGather split point (rows per core)

    perm_rows = np.full((NCORES, nrows), -1, dtype=np.int64)
    blk_k = np.zeros((NCORES, nblk), dtype=np.int64)
    for c in range(NCORES):
        lo = c * npc
        loc = np.argsort(-K_of[lo : lo + npc], kind="stable")
        perm_rows[c, :npc] = lo + loc
        kk = K_of[lo + loc]
        for b in range(nblk):
            seg = kk[b * 128 : (b + 1) * 128]
            blk_k[c, b] = int(seg.max()) if len(seg) else 4
    sched = [int(blk_k[:, b].max()) for b in range(nblk)]
    tot_slots = 128 * sum(sched)

    # pos_of[n] = permuted row index of node n within its core
    pos_of = np.zeros(n_nodes, dtype=np.int64)
    for c in range(NCORES):
        real = perm_rows[c] >= 0
        pos_of[perm_rows[c][real]] = np.nonzero(real)[0]
    # global table row of node n: the AllGather runs in two halves, so
    # the table layout is [cores x first-half | cores x second-half]
    core_of = np.arange(n_nodes) // npc
    tab_of = np.where(
        pos_of < half,
        core_of * half + pos_of,
        NCORES * half + core_of * (nrows - half) + (pos_of - half),
    )

    ssrc_cores = []
    for c in range(NCORES):
        parts = []
        for b in range(nblk):
            K = sched[b]
            nk = perm_rows[c, b * 128 : (b + 1) * 128]
            real = nk >= 0
            nn = np.where(real, nk, 0)
            p = starts[nn][:, None] + np.arange(K)[None, :]
            m = (np.arange(K)[None, :] < deg[nn][:, None]) & real[:, None]
            v = ssrc_sorted[np.clip(p, 0, len(ssrc_sorted) - 1)]
            parts.append(np.where(m, tab_of[v], dummy).reshape(-1))
        arr = np.concatenate(parts).astype(np.int32)
        assert len(arr) == tot_slots
        ssrc_cores.append(arr)

    return dict(
        npc=npc,
        sched=sched,
        nblk=nblk,
        nrows=nrows,
        half=half,
        tot_slots=tot_slots,
        perm_rows=perm_rows,
        ssrc=ssrc_cores,
    )


# ------------------------------------------------------------ device program
def _fold_add(nc, view, K, inner):
    """In-place fold-tree sum over the K axis of an SBUF AP view
    [128, K, inner]; result lands in view[:, 0, :]."""
    cur = K
    while cur > 1:
        half = 1 << (cur.bit_length() - 1)  # largest power of two <= cur
        if half == cur:
            half //= 2
        rem = cur - half  # rem <= half
        nc.vector.tensor_tensor(
            out=view[:, 0:rem, :],
            in0=view[:, 0:rem, :],
            in1=view[:, half : half + rem, :],
            op=OP.add,
        )
        cur = half


def _build_program(meta, dims):
    in_dim = dims["IN"]
    nrows, nblk = meta["nrows"], meta["nblk"]
    sched, tot_slots = meta["sched"], meta["tot_slots"]
    half = meta["half"]
    rest = nrows - half
    HC, H, C = 64, 4, 16
    FF = HC + H  # 68: xw row | a_s row
    NTAB = NCORES * nrows + 1  # + dummy row

    nc = bacc.Bacc(
        "TRN2", target_bir_lowering=False, debug=False, num_devices=NCORES
    )

    # ---- I/O (everything node-indexed is in block-permuted order)
    xT = nc.dram_tensor("xT", [in_dim, nrows], F32, kind="ExternalInput")
    ssrc = nc.dram_tensor("ssrc", [tot_slots], I32, kind="ExternalInput")
    w0_in = nc.dram_tensor("w0", [in_dim, HC], F32, kind="ExternalInput")
    w1_in = nc.dram_tensor("w1", [HC, HC], F32, kind="ExternalInput")
    lw_in = nc.dram_tensor("lw", [2 * HC, HC], F32, kind="ExternalInput")
    br = {}
    for nm in ("asb0", "adb0", "asb1", "adb1", "b0b", "b1b", "lbb"):
        br[nm] = nc.dram_tensor(nm, [128, HC], F32, kind="ExternalInput")

    h0_out = nc.dram_tensor("h0_out", [nrows, HC], F32, kind="ExternalOutput")
    h1_out = nc.dram_tensor("h1_out", [nrows, HC], F32, kind="ExternalOutput")
    lg_out = nc.dram_tensor("lg_out", [nrows, HC], F32, kind="ExternalOutput")
    pr_out = nc.dram_tensor("pr_out", [nrows, HC], F32, kind="ExternalOutput")

    # ---- internal DRAM
    xwp0_own = nc.dram_tensor("xwp0_own", [nrows, FF], F32)
    ad0_tab = nc.dram_tensor("ad0_tab", [nrows, H], F32)
    xwp0_tab = nc.dram_tensor("xwp0_tab", [NTAB, FF], F32, addr_space="Shared")
    xwp1_own = nc.dram_tensor("xwp1_own", [nrows, FF], F32)
    ad1_tab = nc.dram_tensor("ad1_tab", [nrows, H], F32)
    xwp1_tab = nc.dram_tensor("xwp1_tab", [NTAB, FF], F32, addr_space="Shared")

    with tile.TileContext(nc, num_cores=NCORES) as tc:
        with (
            tc.tile_pool(name="const", bufs=1) as cpool,
            tc.tile_pool(name="io", bufs=5) as iop,
            tc.tile_pool(name="gat", bufs=3) as gp,
            tc.tile_pool(name="small", bufs=4) as sp,
            tc.tile_pool(name="idxp", bufs=6) as ixp,
            tc.tile_pool(name="psum", bufs=2, space="PSUM") as pp,
        ):
            # constants
            ident = cpool.tile([128, 128], F32, name="ident")
            make_identity(nc, ident[:])
            w0_t = cpool.tile([in_dim, HC], F32, name="w0_t")
            nc.sync.dma_start(out=w0_t[:], in_=w0_in[:, :])
            w1_t = cpool.tile([HC, HC], F32, name="w1_t")
            nc.sync.dma_start(out=w1_t[:], in_=w1_in[:, :])
            lw_t = cpool.tile([2 * HC, HC], F32, name="lw_t")
            nc.sync.dma_start(out=lw_t[:], in_=lw_in[:, :])
            brt = {}
            for nm, hnd in br.items():
                brt[nm] = cpool.tile([128, HC], F32, name=f"{nm}_t")
                nc.sync.dma_start(out=brt[nm][:], in_=hnd[:, :])
            # dummy table row: zeros except a_s = -1e30
            drow = cpool.tile([1, FF], F32, name="drow")
            nc.vector.memset(drow[:, :HC], 0.0)
            nc.vector.memset(drow[:, HC:FF], NEG_BIG)
            nc.sync.dma_start(out=xwp0_tab[NTAB - 1 : NTAB, :], in_=drow[:, :])
            nc.sync.dma_start(out=xwp1_tab[NTAB - 1 : NTAB, :], in_=drow[:, :])

            def build_rows(src_rows_ap, kdim, asb, adb, xwp_dst, ad_dst, w_t,
                           transpose_first):
                """[128, kdim] node rows -> xw|a_s rows + a_d rows."""
                if transpose_first:
                    rows_t = iop.tile([128, kdim], F32, tag="rows_t")
                    nc.sync.dma_start(out=rows_t[:, :], in_=src_rows_ap)
                    tp = pp.tile([kdim, 128], F32, tag="tp")
                    nc.tensor.transpose(
                        out=tp[:, :], in_=rows_t[:, :], identity=ident[:]
                    )
                    lhsT = iop.tile([kdim, 128], F32, tag="lhsT")
                    nc.vector.tensor_copy(out=lhsT[:, :], in_=tp[:, :])
                else:
                    lhsT = iop.tile([kdim, 128], F32, tag="lhsT")
                    nc.sync.dma_start(out=lhsT[:, :], in_=src_rows_ap)
                xw = pp.tile([128, HC], F32, tag="xw")
                nc.tensor.matmul(
                    out=xw[:, :], lhsT=lhsT[:, :], rhs=w_t[:, :],
                    start=True, stop=True,
                )
                xwp = iop.tile([128, FF], F32, tag="xwp")
                nc.vector.tensor_copy(out=xwp[:, :HC], in_=xw[:, :])
                prod = iop.tile([128, HC], F32, tag="prod")
                nc.vector.tensor_tensor(
                    out=prod[:, :], in0=xw[:, :], in1=asb[:, :], op=OP.mult
                )
                nc.vector.tensor_reduce(
                    out=xwp[:, HC:FF],
                    in_=prod[:, :].rearrange("p (h c) -> p h c", h=H),
                    axis=AX.X,
                    op=OP.add,
                )
                nc.vector.tensor_tensor(
                    out=prod[:, :], in0=xw[:, :], in1=adb[:, :], op=OP.mult
                )
                adt = iop.tile([128, H], F32, tag="adt")
                nc.vector.tensor_reduce(
                    out=adt[:, :],
                    in_=prod[:, :].rearrange("p (h c) -> p h c", h=H),
                    axis=AX.X,
                    op=OP.add,
                )
                nc.sync.dma_start(out=xwp_dst, in_=xwp[:, :])
                nc.sync.dma_start(out=ad_dst, in_=adt[:, :])

            # ---------------- phase A: xwp0 = [x@W0 | a_s0], a_d0 (permuted)
            for i in range(nblk):
                r0 = i * 128
                build_rows(
                    xT[:, r0 : r0 + 128], in_dim, brt["asb0"], brt["adb0"],
                    xwp0_own[r0 : r0 + 128, :], ad0_tab[r0 : r0 + 128, :],
                    w0_t, transpose_first=False,
                )

            def allgather(own, tab):
                """AllGather in two halves so the first half overlaps the
                producing phase's tail; table layout is
                [cores x first-half | cores x second-half]."""
                if half > 0:
                    nc.gpsimd.collective_compute(
                        "AllGather",
                        OP.bypass,
                        replica_groups=[list(range(NCORES))],
                        ins=[own[0:half, :].opt()],
                        outs=[tab[0 : NCORES * half, :].opt()],
                    )
                nc.gpsimd.collective_compute(
                    "AllGather",
                    OP.bypass,
                    replica_groups=[list(range(NCORES))],
                    ins=[own[half:nrows, :].opt()],
                    outs=[
                        tab[NCORES * half : NCORES * half + NCORES * rest, :].opt()
                    ],
                )

            allgather(xwp0_own, xwp0_tab)

            # ---------------- edge aggregation (shared between layers)
            def edge_pass(xwp_tab_h, ad_tab_h, bias_t, h_dst, mid_cbs=None):
                slot_base = 0
                for b, K in enumerate(sched):
                    if mid_cbs and b in mid_cbs:
                        mid_cbs[b]()
                    r0 = b * 128
                    idx = ixp.tile([128, K], I32, tag="idx")
                    nc.sync.dma_start(
                        out=idx[:],
                        in_=ssrc[slot_base : slot_base + 128 * K].rearrange(
                            "(p k) -> p k", p=128
                        ),
                    )
                    g = gp.tile([128, K, FF], F32, tag="g", bufs=4)
                    for k in range(K):
                        nc.gpsimd.indirect_dma_start(
                            out=g[:, k, :],
                            out_offset=None,
                            in_=xwp_tab_h[:, :],
                            in_offset=IndirectOffsetOnAxis(
                                ap=idx[:, k : k + 1], axis=0
                            ),
                        )
                    ad = sp.tile([128, H], F32, tag="ad")
                    nc.sync.dma_start(out=ad[:], in_=ad_tab_h[r0 : r0 + 128, :])
                    # e = a_s[src] + a_d[dst]  -> [128, K, H]
                    e = gp.tile([128, K, H], F32, tag="e")
                    nc.vector.tensor_tensor(
                        out=e[:, :, :],
                        in0=g[:, :, HC:FF],
                        in1=ad[:, None, :].to_broadcast([128, K, H]),
                        op=OP.add,
                    )
                    # ex = exp(leaky_relu(e)) = max(exp(e), exp(0.2*e))
                    ex1 = gp.tile([128, K, H], F32, tag="ex1")
                    nc.scalar.activation(
                        out=ex1[:, :, :], in_=e[:, :, :], func=ACT.Exp
                    )
                    nc.scalar.activation(
                        out=e[:, :, :], in_=e[:, :, :], func=ACT.Exp,
                        scale=NEG_SLOPE,
                    )
                    nc.vector.tensor_tensor(
                        out=ex1[:, :, :], in0=ex1[:, :, :], in1=e[:, :, :],
                        op=OP.max,
                    )
                    # msg = xw[src] * ex  -> [128, K, H, C]
                    msg = gp.tile([128, K, H, C], F32, tag="msg")
                    nc.vector.tensor_tensor(
                        out=msg[:, :, :, :],
                        in0=g[:, :, :HC].rearrange("p k (h c) -> p k h c", h=H),
                        in1=ex1[:, :, :, None].to_broadcast([128, K, H, C]),
                        op=OP.mult,
                    )
                    _fold_add(nc, ex1[:, :, :], K, H)
                    _fold_add(nc, msg[:].rearrange("p k h c -> p k (h c)"), K, HC)
                    rden = sp.tile([128, H], F32, tag="rden")
                    nc.vector.tensor_scalar_add(rden[:, :], ex1[:, 0, :], 1e-16)
                    nc.vector.reciprocal(rden[:, :], rden[:, :])
                    outt = sp.tile([128, HC], F32, tag="outt")
                    nc.vector.tensor_tensor(
                        out=outt[:].rearrange("p (h c) -> p h c", h=H),
                        in0=msg[:, 0, :, :],
                        in1=rden[:, :, None].to_broadcast([128, H, C]),
                        op=OP.mult,
                    )
                    nc.vector.tensor_tensor(
                        out=outt[:, :], in0=outt[:, :], in1=bias_t[:, :],
                        op=OP.add,
                    )
                    # elu(x) = max(x,0) + exp(min(x,0)) - 1
                    mn = sp.tile([128, HC], F32, tag="mn")
                    nc.vector.tensor_scalar_min(mn[:, :], outt[:, :], 0.0)
                    nc.scalar.activation(out=mn[:, :], in_=mn[:, :], func=ACT.Exp)
                    nc.vector.tensor_scalar_max(outt[:, :], outt[:, :], 0.0)
                    nc.vector.tensor_tensor(
                        out=outt[:, :], in0=outt[:, :], in1=mn[:, :], op=OP.add
                    )
                    ht = sp.tile([128, HC], F32, tag="ht")
                    nc.vector.tensor_scalar_add(ht[:, :], outt[:, :], -1.0)
                    nc.sync.dma_start(out=h_dst[r0 : r0 + 128, :], in_=ht[:, :])
                    slot_base += 128 * K

            # phase C chunk: xwp1 = [h0@W1 | a_s1], a_d1 (permuted)
            def phase_c(i):
                r0 = i * 128
                build_rows(
                    h0_out[r0 : r0 + 128, :], HC, brt["asb1"], brt["adb1"],
                    xwp1_own[r0 : r0 + 128, :], ad1_tab[r0 : r0 + 128, :],
                    w1_t, transpose_first=True,
                )

            # emit the first half of phase C + the first AG1 half in the
            # middle of the layer-0 edge pass: the collective then runs on
            # the CC cores while the remaining layer-0 gathers keep Q7 busy
            def mid_cb():
                for i in range(nblk // 2):
                    phase_c(i)
                if half > 0:
                    nc.gpsimd.collective_compute(
                        "AllGather",
                        OP.bypass,
                        replica_groups=[list(range(NCORES))],
                        ins=[xwp1_own[0:half, :].opt()],
                        outs=[xwp1_tab[0 : NCORES * half, :].opt()],
                    )

            mid_at = min(nblk - 1, nblk // 2 + 6)
            edge_pass(
                xwp0_tab, ad0_tab, brt["b0b"], h0_out, mid_cbs={mid_at: mid_cb}
            )

            for i in range(nblk // 2, nblk):
                phase_c(i)
            nc.gpsimd.collective_compute(
                "AllGather",
                OP.bypass,
                replica_groups=[list(range(NCORES))],
                ins=[xwp1_own[half:nrows, :].opt()],
                outs=[
                    xwp1_tab[NCORES * half : NCORES * half + NCORES * rest, :].opt()
                ],
            )

            edge_pass(xwp1_tab, ad1_tab, brt["b1b"], h1_out)

            # ---------------- phase D: logits + softmax (permuted order)
            for i in range(nblk):
                r0 = i * 128
                embT = iop.tile([128, 128], F32, tag="embT")
                for j, h_src in enumerate((h0_out, h1_out)):
                    hc_t = iop.tile([128, HC], F32, tag="hc_t")
                    nc.sync.dma_start(out=hc_t[:], in_=h_src[r0 : r0 + 128, :])
                    tp = pp.tile([HC, 128], F32, tag="tp")
                    nc.tensor.transpose(
                        out=tp[:, :], in_=hc_t[:, :], identity=ident[:]
                    )
                    nc.vector.tensor_copy(
                        out=embT[j * HC : (j + 1) * HC, :], in_=tp[:, :]
                    )
                lg_p = pp.tile([128, HC], F32, tag="lg_p")
                nc.tensor.matmul(
                    out=lg_p[:, :], lhsT=embT[:, :], rhs=lw_t[:, :],
                    start=True, stop=True,
                )
                lg = sp.tile([128, HC], F32, tag="lg")
                nc.vector.tensor_tensor(
                    out=lg[:, :], in0=lg_p[:, :], in1=brt["lbb"][:, :], op=OP.add
                )
                nc.sync.dma_start(out=lg_out[r0 : r0 + 128, :], in_=lg[:, :])
                mx = sp.tile([128, 1], F32, tag="mx")
                nc.vector.tensor_reduce(
                    out=mx[:, :], in_=lg[:, :], axis=AX.X, op=OP.max
                )
                pr = sp.tile([128, HC], F32, tag="pr")
                nc.vector.tensor_scalar(
                    out=pr[:, :], in0=lg[:, :], scalar1=mx[:, :1], scalar2=None,
                    op0=OP.subtract,
                )
                nc.scalar.activation(out=pr[:, :], in_=pr[:, :], func=ACT.Exp)
                sm = sp.tile([128, 1], F32, tag="sm")
                nc.vector.tensor_reduce(
                    out=sm[:, :], in_=pr[:, :], axis=AX.X, op=OP.add
                )
                nc.vector.reciprocal(sm[:, :], sm[:, :])
                nc.vector.tensor_scalar(
                    out=pr[:, :], in0=pr[:, :], scalar1=sm[:, :1], scalar2=None,
                    op0=OP.mult,
                )
                nc.sync.dma_start(out=pr_out[r0 : r0 + 128, :], in_=pr[:, :])

    nc.compile()
    return nc


# ----------------------------------------------------------------- execution
_PROGRAM_CACHE = {}


def _get_program(edge_index, n_nodes, in_dim):
    import hashlib

    ei = np.ascontiguousarray(np.asarray(edge_index, np.int32))
    key = (n_nodes, in_dim, hashlib.sha1(ei.tobytes()).hexdigest())
    if key not in _PROGRAM_CACHE:
        meta = _make_schedule(ei, n_nodes)
        nc = _build_program(meta, {"N": n_nodes, "IN": in_dim})
        _PROGRAM_CACHE[key] = (meta, nc)
    return _PROGRAM_CACHE[key]


def run_gat(x, edge_index, W0, as0, ad0, b0, W1, as1, ad1, b1, lw, lb,
            trace=False):
    n_nodes, in_dim = x.shape
    meta, nc = _get_program(edge_index, n_nodes, in_dim)

    nrows = meta["nrows"]
    rep = lambda v: np.tile(np.asarray(v, np.float32).reshape(1, -1), (128, 1))
    x = np.asarray(x, np.float32)
    in_maps = []
    for c in range(NCORES):
        rows = meta["perm_rows"][c]
        xp = np.zeros((nrows, in_dim), np.float32)
        real = rows >= 0
        xp[real] = x[rows[real]]
        in_maps.append(
            {
                "xT": np.ascontiguousarray(xp.T),
                "ssrc": meta["ssrc"][c],
                "w0": np.asarray(W0, np.float32),
                "w1": np.asarray(W1, np.float32),
                "lw": np.asarray(lw, np.float32),
                "asb0": rep(as0),
                "adb0": rep(ad0),
                "asb1": rep(as1),
                "adb1": rep(ad1),
                "b0b": rep(b0),
                "b1b": rep(b1),
                "lbb": rep(lb),
            }
        )

    res = None
    for attempt in range(3):
        try:
            res = run_bass_kernel_spmd(
                nc, in_maps, core_ids=list(range(NCORES)), trace=trace
            )
            break
        except Exception:
            if attempt == 2:
                raise
            import time as _time

            _time.sleep(5.0)

    HC = 64
    h0 = np.empty((n_nodes, HC), np.float32)
    h1 = np.empty((n_nodes, HC), np.float32)
    lg = np.empty((n_nodes, HC), np.float32)
    pr = np.empty((n_nodes, HC), np.float32)
    for c in range(NCORES):
        rows = meta["perm_rows"][c]
        real = rows >= 0
        nid = rows[real]
        pos = np.nonzero(real)[0]
        r = res.results[c]
        h0[nid] = r["h0_out"][pos]
        h1[nid] = r["h1_out"][pos]
        lg[nid] = r["lg_out"][pos]
        pr[nid] = r["pr_out"][pos]

    views = np.stack([h0, h1])
    final_emb = np.concatenate([h0, h1], axis=1)
    return (views, final_emb, lg, pr), res


def kernel(**inputs):
    outs, _ = run_gat(
        inputs["x"], inputs["edge_index"],
        inputs["W0"], inputs["as0"], inputs["ad0"], inputs["b0"],
        inputs["W1"], inputs["as1"], inputs["ad1"], inputs["b1"],
        inputs["lw"], inputs["lb"],
    )
    return outs
